# revision 8
# baseline (speedup 1.0000x reference)
"""Guided filter (r=40, eps=1e-3) on 8 Trainium2 NeuronCores.

Sharding: pure data-parallel over the batch dim (8 batches -> 8 cores).
Each core processes 3 channel-images of 512x512.

Algorithm per image:
  box2d(x) as two banded matmuls on the TensorEngine (image chunk is the
  stationary operand, the 0/1 band matrix the moving operand; contraction
  runs over the partition dim so each pass both box-filters one axis and
  transposes the layout).
  - V-pass band columns carry 2^round(log2(1/n_h)) (exact in bf16); the
    per-row residual rho_h is folded into stage-B/C scalar slots.
  - W-pass band columns carry bf16(1/n_w).
  Host precomputes bf16(I), bf16(p), bf16(I*p), bf16(I*I) so no on-chip
  converts are needed (p is never needed in f32 at all).
  Stage B frees each PSUM q-bank with a single ACT/DVE read (GPSIMD cannot
  touch PSUM), then chains through SBUF with 2-tensor ops on Pool/DVE.
  V-pass PSUM is drained in [128,1024] 2-bank halves (one ACT + one DVE
  copy per pass); stage-C ra/rb reuse the stage-B q banks.
"""

import os
import sys
import numpy as np
import ml_dtypes
from contextlib import ExitStack

sys.path.insert(0, "/opt/trn_rl_repo")

import concourse.bass as bass
import concourse.tile as tile
from concourse import bacc, mybir
from concourse.bass_utils import run_bass_kernel_spmd

F32 = mybir.dt.float32
BF16 = mybir.dt.bfloat16
ALU = mybir.AluOpType

R = 40
EPS = 1e-3
HW_ = 512
NB = 4  # 128-row blocks per axis
CH = 3  # channels per batch
P = 128
NCORES = 8


_MUL_RECIP_OP = None


def _get_mul_recip_op():
    """Register a fused custom-DVE op: out = Src1 * recip_approx(Src0+C2),
    BITWISE_NOT exponent-flip seed + one inline Newton step (~0.4% rel err,
    one DVE pass instead of reciprocal + tensor_mul)."""
    global _MUL_RECIP_OP
    if _MUL_RECIP_OP is not None:
        return _MUL_RECIP_OP
    import re
    import concourse.dve_ops as dops
    from concourse.dve_spec import AluOp, Bin, C0, C1, C2, Spec, Src0, Src1

    name = "MUL_RECIP_EPS_GF"
    _x = Src0 + C2
    _not_x = Bin(AluOp.BITWISE_NOT, _x, _x)
    _y0 = _not_x * C0

    def _ref(in0, in1, c0, c1, c2):
        x = in0 + c2
        not_x = (~x.view(np.int32)).view(np.float32)
        y0 = not_x * c0
        return in1 * (y0 * (c1 - x * y0))

    op = dops.DveOp(
        name, Spec(body=Src1 * (_y0 * (C1 - _x * _y0)), reference=_ref),
        subdim=False, uops_sha={})
    dops.OPS.append(op)
    dops.CUSTOM_DVE_SPECS[name] = op.spec
    dops._SUB_OPCODE_FOR_NAME[name] = max(dops._SUB_OPCODE_FOR_NAME.values()) + 1
    for ver in ("v3", "v4"):
        try:
            op.compile(ver)
        except ValueError as e:
            m = re.search(r'uops_sha\["%s"\]="([0-9a-f]+)"' % ver, str(e))
            if not m:
                raise
            op.uops_sha[ver] = m.group(1)
            dops._COMPILE_CACHE.pop((name, ver), None)
            op.compile(ver)
    _MUL_RECIP_OP = op
    return op


def _band_range(c):
    n0 = max(0, P * c - R)
    n1 = min(HW_, P * c + P + R)
    return n0, n1


_BAND_OFF = []
_BAND_W = []
_off = 0
for _c in range(NB):
    _n0, _n1 = _band_range(_c)
    _BAND_OFF.append(_off)
    _BAND_W.append(_n1 - _n0)
    _off += _n1 - _n0
BAND_TOT = _off  # 792


def make_consts():
    idx = np.arange(HW_)
    n1d = (np.minimum(idx + R, HW_ - 1) - np.maximum(idx - R, 0) + 1).astype(np.float64)
    inv_n = 1.0 / n1d
    E = np.round(np.log2(inv_n))
    po2 = 2.0 ** E                      # exact in bf16
    rho = (inv_n * 2.0 ** (-E)).astype(np.float32)   # residual, ~[0.7, 1.42]

    mask = (np.abs(idx[:, None] - idx[None, :]) <= R)
    bandV = (mask * po2[None, :]).astype(ml_dtypes.bfloat16)
    bandW = (mask * inv_n[None, :]).astype(ml_dtypes.bfloat16)
    # [512k, 512n] -> [128 kp, NB, 512] then pack only the band cols
    bandV = bandV.reshape(NB, P, HW_).transpose(1, 0, 2)
    bandW = bandW.reshape(NB, P, HW_).transpose(1, 0, 2)

    def pack(b):
        cols = []
        for c in range(NB):
            n0, n1 = _band_range(c)
            cols.append(b[:, c, n0:n1])
        return np.ascontiguousarray(np.concatenate(cols, axis=1))

    rho_t = np.ascontiguousarray(rho.reshape(NB, P).T)          # [128, NB]
    return {"bandV": pack(bandV), "bandW": pack(bandW), "rho": rho_t}


def _img_view(dram_ap, c):
    # [3, 512, 512] DRAM tensor -> channel c as [128 hp, 4 hb, 512 w]
    return dram_ap[c].rearrange("(hb hp) w -> hp hb w", hp=P)


def _sb3(t):
    # [128, 2048] SBUF tile AP -> [128, 4, 512]
    return t[:].rearrange("p (hb w) -> p hb w", w=HW_)


def build_model():
    nc = bacc.Bacc("TRN2", target_bir_lowering=False, debug=False,
                   num_devices=NCORES)
    I_d = nc.dram_tensor("I", [CH, HW_, HW_], F32, kind="ExternalInput").ap()
    Ibf_d = nc.dram_tensor("Ibf", [CH, HW_, HW_], BF16, kind="ExternalInput").ap()
    pbf_d = nc.dram_tensor("pbf", [CH, HW_, HW_], BF16, kind="ExternalInput").ap()
    Ipbf_d = nc.dram_tensor("Ipbf", [CH, HW_, HW_], BF16, kind="ExternalInput").ap()
    IIbf_d = nc.dram_tensor("IIbf", [CH, HW_, HW_], BF16, kind="ExternalInput").ap()
    bandV_d = nc.dram_tensor("bandV", [P, BAND_TOT], BF16, kind="ExternalInput").ap()
    bandW_d = nc.dram_tensor("bandW", [P, BAND_TOT], BF16, kind="ExternalInput").ap()
    rho_d = nc.dram_tensor("rho", [P, NB], F32, kind="ExternalInput").ap()
    out_d = nc.dram_tensor("out", [CH, HW_, HW_], F32, kind="ExternalOutput").ap()

    with tile.TileContext(nc) as tc:
        with ExitStack() as ctx:
            build_kernel(ctx, tc, I_d, Ibf_d, pbf_d, Ipbf_d, IIbf_d, out_d,
                         bandV_d, bandW_d, rho_d)
    nc.compile()
    return nc


def build_kernel(ctx, tc, I_d, Ibf_d, pbf_d, Ipbf_d, IIbf_d, out_d,
                 bandV_d, bandW_d, rho_d):
    nc = tc.nc
    FW = NB * HW_  # 2048

    # start the first image's bf16 input DMAs before the band consts
    pBf = ctx.enter_context(tc.tile_pool(name="ibf", bufs=2))
    pIf = ctx.enter_context(tc.tile_pool(name="If", bufs=2))
    in0 = {}
    for tag, d in (("Ibf", Ibf_d), ("pbf", pbf_d), ("Ipbf", Ipbf_d),
                   ("IIbf", IIbf_d)):
        t = pBf.tile([P, FW], BF16, tag=tag, name=tag)
        nc.sync.dma_start(_sb3(t), _img_view(d, 0))
        in0[tag] = t

    consts = ctx.enter_context(tc.tile_pool(name="consts", bufs=1))
    bandV = consts.tile_from(bandV_d)
    bandW = consts.tile_from(bandW_d)
    rho = consts.tile_from(rho_d)

    If0 = pIf.tile([P, FW], F32, tag="If", name="If")
    nc.sync.dma_start(_sb3(If0), _img_view(I_d, 0))

    pY = ctx.enter_context(tc.tile_pool(name="ymid", bufs=2))
    pAB = ctx.enter_context(tc.tile_pool(name="ab", bufs=2))
    pOut = ctx.enter_context(tc.tile_pool(name="outp", bufs=2))
    pT = ctx.enter_context(tc.tile_pool(name="tmps", bufs=2))
    pV = ctx.enter_context(tc.tile_pool(name="psv", bufs=2, space="PSUM"))
    pQ = ctx.enter_context(tc.tile_pool(name="psq", bufs=1, space="PSUM"))

    def vpass(src_bf, dst_bf, engines):
        """One banded V-pass: src [h|w] bf16 -> dst [w|h] bf16 (box over
        partition axis + transpose). 16 MMs into two 2-bank PSUM tiles,
        each drained by one [128,1024] copy (engine per-half)."""
        for half in range(2):
            ps = pV.tile([P, 2 * HW_], F32, tag="ps")
            for i2 in range(2):
                i = 2 * half + i2
                for j in range(NB):
                    n0, n1 = _band_range(j)
                    nc.tensor.matmul(
                        ps[:, i2 * HW_ + n0: i2 * HW_ + n1],
                        lhsT=src_bf[:, j * HW_ + i * P: j * HW_ + i * P + P],
                        rhs=bandV[:, _BAND_OFF[j]: _BAND_OFF[j] + _BAND_W[j]],
                        start=(j == 0), stop=(j == NB - 1))
            dst = dst_bf[:, 2 * half * HW_: 2 * (half + 1) * HW_]
            if engines[half] == "dve":
                nc.vector.tensor_copy(dst, ps[:])
            else:
                nc.scalar.copy(dst, ps[:])

    def wpass_mm(src_bf, q_tile, j):
        """W-direction banded MMs for output h-chunk j into q_tile."""
        for i in range(NB):
            m0, m1 = _band_range(i)
            nc.tensor.matmul(
                q_tile[:, m0:m1],
                lhsT=src_bf[:, i * HW_ + j * P: i * HW_ + j * P + P],
                rhs=bandW[:, _BAND_OFF[i]: _BAND_OFF[i] + _BAND_W[i]],
                start=(i == 0), stop=(i == NB - 1))

    def stageA(c):
        """DMA + stage-1 V-passes for image c."""
        if c == 0:
            I_bf, p_bf = in0["Ibf"], in0["pbf"]
            Ip_bf, II_bf = in0["Ipbf"], in0["IIbf"]
            I_f = If0
        else:
            I_bf = pBf.tile([P, FW], BF16, tag="Ibf", name="Ibf")
            p_bf = pBf.tile([P, FW], BF16, tag="pbf", name="pbf")
            Ip_bf = pBf.tile([P, FW], BF16, tag="Ipbf", name="Ipbf")
            II_bf = pBf.tile([P, FW], BF16, tag="IIbf", name="IIbf")
            I_f = pIf.tile([P, FW], F32, tag="If", name="If")
            nc.sync.dma_start(_sb3(I_bf), _img_view(Ibf_d, c))
            nc.sync.dma_start(_sb3(p_bf), _img_view(pbf_d, c))
            nc.sync.dma_start(_sb3(Ip_bf), _img_view(Ipbf_d, c))
            nc.sync.dma_start(_sb3(II_bf), _img_view(IIbf_d, c))
            nc.sync.dma_start(_sb3(I_f), _img_view(I_d, c))

        yI = pY.tile([P, FW], BF16, tag="yI", name="yI")
        yp = pY.tile([P, FW], BF16, tag="yp", name="yp")
        yIp = pY.tile([P, FW], BF16, tag="yIp", name="yIp")
        yII = pY.tile([P, FW], BF16, tag="yII", name="yII")
        vpass(I_bf, yI, ["act", "dve"])
        vpass(p_bf, yp, ["act", "dve"])
        vpass(Ip_bf, yIp, ["act", "dve"])
        vpass(II_bf, yII, ["act", "dve"])
        return dict(I_f=I_f, yI=yI, yp=yp, yIp=yIp, yII=yII)

    def stageB(st):
        """Stage-2 W-passes + elementwise -> a, b for image state st.

        Each q PSUM bank is freed by a single ACT/DVE read (rho folded into
        the scalar slot); the rest of the chain runs out of SBUF with the
        2-tensor ops split between Pool and DVE."""
        yI, yp, yIp, yII = st["yI"], st["yp"], st["yIp"], st["yII"]
        a_bf = pAB.tile([P, FW], BF16, tag="abf", name="abf")
        b_bf = pAB.tile([P, FW], BF16, tag="bbf", name="bbf")
        st["a_bf"], st["b_bf"] = a_bf, b_bf
        for j in range(NB):
            qI = pQ.tile([P, HW_], F32, tag="qI")
            qp = pQ.tile([P, HW_], F32, tag="qp")
            qIp = pQ.tile([P, HW_], F32, tag="qIp")
            qII = pQ.tile([P, HW_], F32, tag="qII")
            wpass_mm(yI, qI, j)
            wpass_mm(yp, qp, j)
            wpass_mm(yIp, qIp, j)
            wpass_mm(yII, qII, j)

            s = rho[:, j:j + 1]
            sl = slice(j * HW_, (j + 1) * HW_)
            mI = pT.tile([P, HW_], F32, tag="mI")
            mp = pT.tile([P, HW_], F32, tag="mp")
            cIp = pT.tile([P, HW_], F32, tag="cIp")
            v = pT.tile([P, HW_], F32, tag="v")
            u = pT.tile([P, HW_], F32, tag="u")
            cov = pT.tile([P, HW_], F32, tag="cov")
            den = pT.tile([P, HW_], F32, tag="den")
            t = pT.tile([P, HW_], F32, tag="t")
            # free the four q banks fast: one PSUM read each (ACT/DVE only)
            nc.scalar.mul(mI[:], qI[:], s)                       # ACT
            nc.scalar.mul(mp[:], qp[:], s)                       # ACT
            nc.vector.tensor_scalar_mul(cIp[:], qIp[:], s)      # DVE
            v_ = v[:]
            nc.scalar.activation(v_, mI[:],
                                 mybir.ActivationFunctionType.Square)  # ACT
            u_ = u[:]
            nc.gpsimd.tensor_tensor(u_, mI[:], mp[:], op=ALU.mult)  # Pool
            nc.vector.scalar_tensor_tensor(
                den[:], qII[:], s, v_, op0=ALU.mult, op1=ALU.subtract)  # DVE
            nc.vector.tensor_tensor(cov[:], cIp[:], u_, op=ALU.subtract)  # DVE
            nc.vector._custom_dve(
                _get_mul_recip_op(), out=a_bf[:, sl], in0=den[:], in1=cov[:],
                s0=-0.23549792, s1=2.0017324, imm2=EPS)
            nc.gpsimd.tensor_tensor(t[:], mI[:], a_bf[:, sl], op=ALU.mult)
            nc.gpsimd.tensor_tensor(b_bf[:, sl], mp[:], t[:], op=ALU.subtract)

    def stageC(c, st):
        """Stage-3 box2d(a), box2d(b) + combine + output DMA. ra/rb reuse
        the stage-B q banks (pQ pool)."""
        a_bf, b_bf, I_f = st["a_bf"], st["b_bf"], st["I_f"]
        ya = pY.tile([P, FW], BF16, tag="ya", name="ya")
        yb = pY.tile([P, FW], BF16, tag="yb", name="yb")
        vpass(a_bf, ya, ["act", "dve"])
        vpass(b_bf, yb, ["act", "dve"])

        out_t = pOut.tile([P, FW], F32, tag="out", name="out")
        for j in range(NB):
            ra = pQ.tile([P, HW_], F32, tag="qI", name="ra")
            rb = pQ.tile([P, HW_], F32, tag="qp", name="rb")
            wpass_mm(ya, ra, j)
            wpass_mm(yb, rb, j)
            s = rho[:, j:j + 1]
            sl = slice(j * HW_, (j + 1) * HW_)
            f1 = pT.tile([P, HW_], F32, tag="f1", name="f1")
            nc.vector.scalar_tensor_tensor(
                f1[:], ra[:], s, I_f[:, sl], op0=ALU.mult, op1=ALU.mult)
            nc.vector.scalar_tensor_tensor(
                out_t[:, sl], rb[:], s, f1[:], op0=ALU.mult, op1=ALU.add)
            nc.sync.dma_start(_img_view(out_d, c)[:, j, :], out_t[:, sl])

    # software pipeline: PE always has independent V-pass work queued
    # while the previous image's elementwise chain drains.
    st0 = stageA(0)
    stageB(st0)
    st1 = stageA(1)
    stageC(0, st0)
    stageB(st1)
    st2 = stageA(2)
    stageC(1, st1)
    stageB(st2)
    stageC(2, st2)


_NC_CACHE = None
LAST_RESULT = None


def _get_model():
    global _NC_CACHE
    if _NC_CACHE is None:
        _NC_CACHE = build_model()
    return _NC_CACHE


def kernel(I, p):
    global LAST_RESULT
    I = np.asarray(I, dtype=np.float32)
    p = np.asarray(p, dtype=np.float32)
    B = I.shape[0]
    assert I.shape == (B, CH, HW_, HW_), I.shape
    nc = _get_model()
    consts = make_consts()
    I_bf = I.astype(ml_dtypes.bfloat16)
    p_bf = p.astype(ml_dtypes.bfloat16)
    Ip_bf = (I_bf.astype(np.float32) * p_bf.astype(np.float32)).astype(
        ml_dtypes.bfloat16)
    II_bf = (I * I).astype(ml_dtypes.bfloat16)
    in_maps = []
    for k in range(NCORES):
        m = {"I": np.ascontiguousarray(I[k]),
             "Ibf": np.ascontiguousarray(I_bf[k]),
             "pbf": np.ascontiguousarray(p_bf[k]),
             "Ipbf": np.ascontiguousarray(Ip_bf[k]),
             "IIbf": np.ascontiguousarray(II_bf[k])}
        m.update(consts)
        in_maps.append(m)
    kwargs = {}
    if os.environ.get("BASS_TRACE_DIR"):
        kwargs["tmpdir"] = os.environ["BASS_TRACE_DIR"]
    res = run_bass_kernel_spmd(nc, in_maps, core_ids=list(range(NCORES)), **kwargs)
    LAST_RESULT = res
    out = np.stack([res.results[k]["out"] for k in range(NCORES)], axis=0)
    return out.astype(np.float32)


if __name__ == "__main__":
    rng = np.random.default_rng(0)
    I = rng.random((8, CH, HW_, HW_), dtype=np.float32)
    p = rng.random((8, CH, HW_, HW_), dtype=np.float32)
    out = kernel(I, p)
    print("out", out.shape, out.dtype, float(out.mean()))


# revision 9
# speedup vs baseline: 1.0377x; 1.0377x over previous
"""Guided filter (r=40, eps=1e-3) on 8 Trainium2 NeuronCores.

Sharding: pure data-parallel over the batch dim (8 batches -> 8 cores).
Each core processes 3 channel-images of 512x512.

Algorithm per image:
  box2d(x) as two banded matmuls on the TensorEngine (image chunk is the
  stationary operand, the 0/1 band matrix the moving operand; contraction
  runs over the partition dim so each pass both box-filters one axis and
  transposes the layout).
  - V-pass band columns carry 2^round(log2(1/n_h)) (exact in bf16); the
    per-row residual rho_h is folded into stage-B/C scalar slots.
  - W-pass band columns carry bf16(1/n_w).
  Host precomputes bf16(I), bf16(p), bf16(I*p), bf16(I*I), stacked in one
  DRAM tensor so each image's inputs arrive in 2 DMA dispatches (issued
  from the ACT queue so they overlap the const loads on the sync queue).
  Stage B frees each PSUM q-bank with a single ACT/DVE read (GPSIMD cannot
  touch PSUM), then chains through SBUF; V-pass PSUM is drained in
  [128,1024] 2-bank halves (one ACT + one DVE copy per pass); stage-C
  ra/rb reuse the stage-B q banks.  The three images' stages are
  interleaved at ~1.5us unit granularity to keep the PE dense (HAM warm).
"""

import os
import sys
import numpy as np
import ml_dtypes
from contextlib import ExitStack

sys.path.insert(0, "/opt/trn_rl_repo")

import concourse.bass as bass
import concourse.tile as tile
from concourse import bacc, mybir
from concourse.bass_utils import run_bass_kernel_spmd

F32 = mybir.dt.float32
BF16 = mybir.dt.bfloat16
ALU = mybir.AluOpType

R = 40
EPS = 1e-3
HW_ = 512
NB = 4  # 128-row blocks per axis
CH = 3  # channels per batch
NQ = 4  # stacked bf16 quantities: I, p, I*p, I*I
P = 128
NCORES = 8


_MUL_RECIP_OP = None


def _get_mul_recip_op():
    """Register a fused custom-DVE op: out = Src1 * recip_approx(Src0+C2),
    BITWISE_NOT exponent-flip seed + one inline Newton step (~0.4% rel err,
    one DVE pass instead of reciprocal + tensor_mul)."""
    global _MUL_RECIP_OP
    if _MUL_RECIP_OP is not None:
        return _MUL_RECIP_OP
    import re
    import concourse.dve_ops as dops
    from concourse.dve_spec import AluOp, Bin, C0, C1, C2, Spec, Src0, Src1

    name = "MUL_RECIP_EPS_GF"
    _x = Src0 + C2
    _not_x = Bin(AluOp.BITWISE_NOT, _x, _x)
    _y0 = _not_x * C0

    def _ref(in0, in1, c0, c1, c2):
        x = in0 + c2
        not_x = (~x.view(np.int32)).view(np.float32)
        y0 = not_x * c0
        return in1 * (y0 * (c1 - x * y0))

    op = dops.DveOp(
        name, Spec(body=Src1 * (_y0 * (C1 - _x * _y0)), reference=_ref),
        subdim=False, uops_sha={})
    dops.OPS.append(op)
    dops.CUSTOM_DVE_SPECS[name] = op.spec
    dops._SUB_OPCODE_FOR_NAME[name] = max(dops._SUB_OPCODE_FOR_NAME.values()) + 1
    for ver in ("v3", "v4"):
        try:
            op.compile(ver)
        except ValueError as e:
            m = re.search(r'uops_sha\["%s"\]="([0-9a-f]+)"' % ver, str(e))
            if not m:
                raise
            op.uops_sha[ver] = m.group(1)
            dops._COMPILE_CACHE.pop((name, ver), None)
            op.compile(ver)
    _MUL_RECIP_OP = op
    return op


def _band_range(c):
    n0 = max(0, P * c - R)
    n1 = min(HW_, P * c + P + R)
    return n0, n1


_BAND_OFF = []
_BAND_W = []
_off = 0
for _c in range(NB):
    _n0, _n1 = _band_range(_c)
    _BAND_OFF.append(_off)
    _BAND_W.append(_n1 - _n0)
    _off += _n1 - _n0
BAND_TOT = _off  # 792


def make_consts():
    idx = np.arange(HW_)
    n1d = (np.minimum(idx + R, HW_ - 1) - np.maximum(idx - R, 0) + 1).astype(np.float64)
    inv_n = 1.0 / n1d
    E = np.round(np.log2(inv_n))
    po2 = 2.0 ** E                      # exact in bf16
    rho = (inv_n * 2.0 ** (-E)).astype(np.float32)   # residual, ~[0.7, 1.42]

    mask = (np.abs(idx[:, None] - idx[None, :]) <= R)
    bandV = (mask * po2[None, :]).astype(ml_dtypes.bfloat16)
    bandW = (mask * inv_n[None, :]).astype(ml_dtypes.bfloat16)
    # [512k, 512n] -> [128 kp, NB, 512] then pack only the band cols
    bandV = bandV.reshape(NB, P, HW_).transpose(1, 0, 2)
    bandW = bandW.reshape(NB, P, HW_).transpose(1, 0, 2)

    def pack(b):
        cols = []
        for c in range(NB):
            n0, n1 = _band_range(c)
            cols.append(b[:, c, n0:n1])
        return np.ascontiguousarray(np.concatenate(cols, axis=1))

    rho_t = np.ascontiguousarray(rho.reshape(NB, P).T)          # [128, NB]
    return {"bandV": pack(bandV), "bandW": pack(bandW), "rho": rho_t}


def build_model():
    nc = bacc.Bacc("TRN2", target_bir_lowering=False, debug=False,
                   num_devices=NCORES)
    I_d = nc.dram_tensor("I", [CH, HW_, HW_], F32, kind="ExternalInput").ap()
    Q_d = nc.dram_tensor("Qbf", [CH, NQ, HW_, HW_], BF16,
                         kind="ExternalInput").ap()
    bandV_d = nc.dram_tensor("bandV", [P, BAND_TOT], BF16, kind="ExternalInput").ap()
    bandW_d = nc.dram_tensor("bandW", [P, BAND_TOT], BF16, kind="ExternalInput").ap()
    rho_d = nc.dram_tensor("rho", [P, NB], F32, kind="ExternalInput").ap()
    out_d = nc.dram_tensor("out", [CH, HW_, HW_], F32, kind="ExternalOutput").ap()

    with tile.TileContext(nc) as tc:
        with ExitStack() as ctx:
            build_kernel(ctx, tc, I_d, Q_d, out_d, bandV_d, bandW_d, rho_d)
    nc.compile()
    return nc


def build_kernel(ctx, tc, I_d, Q_d, out_d, bandV_d, bandW_d, rho_d):
    nc = tc.nc
    FW = NB * HW_    # 2048 free cols per quantity-image
    QW = NQ * FW     # 8192 free cols for the 4 stacked quantities

    pQin = ctx.enter_context(tc.tile_pool(name="qin", bufs=2))
    pIf = ctx.enter_context(tc.tile_pool(name="If", bufs=2))

    def issue_dma(c, st):
        """Prefetch image c's inputs: 2 half-dispatches of the bf16 stack
        plus the f32 guide, all on the ACT HWDGE queue."""
        Q = pQin.tile([P, QW], BF16, tag="Qbf", name="Qbf")
        I_f = pIf.tile([P, FW], F32, tag="If", name="If")
        for h in range(2):
            dst = Q[:, h * 2 * FW:(h + 1) * 2 * FW].rearrange(
                "p (q hb w) -> p q hb w", q=2, w=HW_)
            src = Q_d[c, 2 * h:2 * h + 2].rearrange(
                "q (hb hp) w -> hp q hb w", hp=P)
            nc.scalar.dma_start(dst, src)
        nc.scalar.dma_start(
            I_f[:].rearrange("p (hb w) -> p hb w", w=HW_),
            I_d[c].rearrange("(hb hp) w -> hp hb w", hp=P))
        st["Q"], st["I_f"] = Q, I_f

    consts = ctx.enter_context(tc.tile_pool(name="consts", bufs=1))
    bandV = consts.tile_from(bandV_d)
    bandW = consts.tile_from(bandW_d)
    rho = consts.tile_from(rho_d)

    pY = ctx.enter_context(tc.tile_pool(name="ymid", bufs=2))
    pAB = ctx.enter_context(tc.tile_pool(name="ab", bufs=2))
    pOut = ctx.enter_context(tc.tile_pool(name="outp", bufs=2))
    pT = ctx.enter_context(tc.tile_pool(name="tmps", bufs=2))
    pV = ctx.enter_context(tc.tile_pool(name="psv", bufs=2, space="PSUM"))
    pQ = ctx.enter_context(tc.tile_pool(name="psq", bufs=1, space="PSUM"))

    def vpass(src_bf, src_off, dst_bf, engines):
        """One banded V-pass: src [h|w] bf16 -> dst [w|h] bf16 (box over
        partition axis + transpose). 16 MMs into two 2-bank PSUM tiles,
        each drained by one [128,1024] copy (engine per-half)."""
        for half in range(2):
            ps = pV.tile([P, 2 * HW_], F32, tag="ps")
            for i2 in range(2):
                i = 2 * half + i2
                for j in range(NB):
                    n0, n1 = _band_range(j)
                    o = src_off + j * HW_ + i * P
                    nc.tensor.matmul(
                        ps[:, i2 * HW_ + n0: i2 * HW_ + n1],
                        lhsT=src_bf[:, o: o + P],
                        rhs=bandV[:, _BAND_OFF[j]: _BAND_OFF[j] + _BAND_W[j]],
                        start=(j == 0), stop=(j == NB - 1))
            dst = dst_bf[:, 2 * half * HW_: 2 * (half + 1) * HW_]
            if engines[half] == "dve":
                nc.vector.tensor_copy(dst, ps[:])
            else:
                nc.scalar.copy(dst, ps[:])

    def wpass_mm(src_bf, q_tile, j):
        """W-direction banded MMs for output h-chunk j into q_tile."""
        for i in range(NB):
            m0, m1 = _band_range(i)
            nc.tensor.matmul(
                q_tile[:, m0:m1],
                lhsT=src_bf[:, i * HW_ + j * P: i * HW_ + j * P + P],
                rhs=bandW[:, _BAND_OFF[i]: _BAND_OFF[i] + _BAND_W[i]],
                start=(i == 0), stop=(i == NB - 1))

    def stageA_units(st):
        """4 units: V-passes for I, p, Ip, II from the stacked input."""
        Q = st["Q"]
        for q, ytag in enumerate(("yI", "yp", "yIp", "yII")):
            y = pY.tile([P, FW], BF16, tag=ytag, name=ytag)
            st[ytag] = y
            vpass(Q, q * FW, y, ["act", "dve"])
            yield

    def stageB_units(st):
        """4 units (one per h-chunk j): W-passes + elementwise -> a, b."""
        a_bf = pAB.tile([P, FW], BF16, tag="abf", name="abf")
        b_bf = pAB.tile([P, FW], BF16, tag="bbf", name="bbf")
        st["a_bf"], st["b_bf"] = a_bf, b_bf
        for j in range(NB):
            qI = pQ.tile([P, HW_], F32, tag="qI")
            qp = pQ.tile([P, HW_], F32, tag="qp")
            qIp = pQ.tile([P, HW_], F32, tag="qIp")
            qII = pQ.tile([P, HW_], F32, tag="qII")
            wpass_mm(st["yI"], qI, j)
            wpass_mm(st["yp"], qp, j)
            wpass_mm(st["yIp"], qIp, j)
            wpass_mm(st["yII"], qII, j)

            s = rho[:, j:j + 1]
            sl = slice(j * HW_, (j + 1) * HW_)
            mI = pT.tile([P, HW_], F32, tag="mI")
            mp = pT.tile([P, HW_], F32, tag="mp")
            cIp = pT.tile([P, HW_], F32, tag="cIp")
            v = pT.tile([P, HW_], F32, tag="v")
            u = pT.tile([P, HW_], F32, tag="u")
            cov = pT.tile([P, HW_], F32, tag="cov")
            den = pT.tile([P, HW_], F32, tag="den")
            t = pT.tile([P, HW_], F32, tag="t")
            # free the four q banks fast: one PSUM read each (ACT/DVE only)
            nc.scalar.mul(mI[:], qI[:], s)                       # ACT
            nc.scalar.mul(mp[:], qp[:], s)                       # ACT
            nc.scalar.mul(cIp[:], qIp[:], s)                     # ACT
            v_ = v[:]
            nc.scalar.activation(v_, mI[:],
                                 mybir.ActivationFunctionType.Square)  # ACT
            u_ = u[:]
            nc.gpsimd.tensor_tensor(u_, mI[:], mp[:], op=ALU.mult)  # Pool
            nc.vector.scalar_tensor_tensor(
                den[:], qII[:], s, v_, op0=ALU.mult, op1=ALU.subtract)  # DVE
            nc.vector.scalar_tensor_tensor(
                cov[:], cIp[:], 1.0, u_, op0=ALU.mult, op1=ALU.subtract)  # DVE
            nc.vector._custom_dve(
                _get_mul_recip_op(), out=a_bf[:, sl], in0=den[:], in1=cov[:],
                s0=-0.23549792, s1=2.0017324, imm2=EPS)
            nc.gpsimd.tensor_tensor(t[:], mI[:], a_bf[:, sl], op=ALU.mult)
            nc.gpsimd.tensor_tensor(b_bf[:, sl], mp[:], t[:], op=ALU.subtract)
            yield

    def stageC_units(c, st):
        """6 units: V(a), V(b), then per-j W-passes + combine + out DMA.
        ra/rb reuse the stage-B q banks (pQ pool)."""
        a_bf, b_bf, I_f = st["a_bf"], st["b_bf"], st["I_f"]
        ya = pY.tile([P, FW], BF16, tag="ya", name="ya")
        yb = pY.tile([P, FW], BF16, tag="yb", name="yb")
        vpass(a_bf, 0, ya, ["act", "dve"])
        yield
        vpass(b_bf, 0, yb, ["act", "dve"])
        yield

        out_t = pOut.tile([P, FW], F32, tag="out", name="out")
        for j in range(NB):
            ra = pQ.tile([P, HW_], F32, tag="qI", name="ra")
            rb = pQ.tile([P, HW_], F32, tag="qp", name="rb")
            wpass_mm(ya, ra, j)
            wpass_mm(yb, rb, j)
            s = rho[:, j:j + 1]
            sl = slice(j * HW_, (j + 1) * HW_)
            f1 = pT.tile([P, HW_], F32, tag="f1", name="f1")
            nc.vector.scalar_tensor_tensor(
                f1[:], ra[:], s, I_f[:, sl], op0=ALU.mult, op1=ALU.mult)
            nc.vector.scalar_tensor_tensor(
                out_t[:, sl], rb[:], s, f1[:], op0=ALU.mult, op1=ALU.add)
            nc.sync.dma_start(
                out_d[c].rearrange("(hb hp) w -> hp hb w", hp=P)[:, j, :],
                out_t[:, sl])
            yield

    # -- fine-grained software pipeline over the 3 images ------------------
    sts = [{}, {}, {}]
    issue_dma(0, sts[0])
    A = [stageA_units(sts[c]) for c in range(CH)]
    B = [stageB_units(sts[c]) for c in range(CH)]
    C = [stageC_units(c, sts[c]) for c in range(CH)]

    def run(gen):
        next(gen)

    for _ in range(4):
        run(A[0])
    issue_dma(1, sts[1])
    for _ in range(4):
        run(B[0]); run(A[1])
    issue_dma(2, sts[2])
    run(C[0]); run(B[1])      # C0.va   B1.j0
    run(C[0]); run(B[1])      # C0.vb   B1.j1
    run(C[0]); run(B[1])      # C0.j0   B1.j2
    run(C[0]); run(B[1])      # C0.j1   B1.j3
    run(C[0]); run(A[2])      # C0.j2   A2.v0
    run(C[0]); run(A[2])      # C0.j3   A2.v1
    run(C[1]); run(A[2])      # C1.va   A2.v2
    run(C[1]); run(A[2])      # C1.vb   A2.v3
    for _ in range(4):
        run(C[1]); run(B[2])  # C1.j*   B2.j*
    for _ in range(6):
        run(C[2])


_NC_CACHE = None
LAST_RESULT = None


def _get_model():
    global _NC_CACHE
    if _NC_CACHE is None:
        _NC_CACHE = build_model()
    return _NC_CACHE


def kernel(I, p):
    global LAST_RESULT
    I = np.asarray(I, dtype=np.float32)
    p = np.asarray(p, dtype=np.float32)
    B = I.shape[0]
    assert I.shape == (B, CH, HW_, HW_), I.shape
    nc = _get_model()
    consts = make_consts()
    I_bf = I.astype(ml_dtypes.bfloat16)
    p_bf = p.astype(ml_dtypes.bfloat16)
    Ip_bf = (I_bf.astype(np.float32) * p_bf.astype(np.float32)).astype(
        ml_dtypes.bfloat16)
    II_bf = (I * I).astype(ml_dtypes.bfloat16)
    Q = np.ascontiguousarray(
        np.stack([I_bf, p_bf, Ip_bf, II_bf], axis=2))  # [B, CH, NQ, H, W]
    in_maps = []
    for k in range(NCORES):
        m = {"I": np.ascontiguousarray(I[k]), "Qbf": Q[k]}
        m.update(consts)
        in_maps.append(m)
    kwargs = {}
    if os.environ.get("BASS_TRACE_DIR"):
        kwargs["tmpdir"] = os.environ["BASS_TRACE_DIR"]
    res = run_bass_kernel_spmd(nc, in_maps, core_ids=list(range(NCORES)), **kwargs)
    LAST_RESULT = res
    out = np.stack([res.results[k]["out"] for k in range(NCORES)], axis=0)
    return out.astype(np.float32)


if __name__ == "__main__":
    rng = np.random.default_rng(0)
    I = rng.random((8, CH, HW_, HW_), dtype=np.float32)
    p = rng.random((8, CH, HW_, HW_), dtype=np.float32)
    out = kernel(I, p)
    print("out", out.shape, out.dtype, float(out.mean()))
